# revision 1
# baseline (speedup 1.0000x reference)
"""CapsNet forward on 8 trn2 NeuronCores — data-parallel convs on device."""
import numpy as np
import ml_dtypes

B = 256
NCORES = 8
BL = B // NCORES          # 32 images per core
POS1 = 32 * 20 * 20       # conv1 output positions per core (img,oh,ow)
K1 = 82                   # 81 taps + 1 bias row
KHW = 81
NPOS2 = 36                # 6x6
CHUNKS = [(0, 12), (12, 12), (24, 8)]

_exec_time_ns = None


def _build_and_run_device(im2col_np, w1t_np, w2_np):
    import concourse.bass as bass
    import concourse.bacc as bacc
    import concourse.mybir as mybir
    import concourse.tile as tile
    from concourse.bass_utils import run_bass_kernel_spmd

    bf16 = mybir.dt.bfloat16
    f32 = mybir.dt.float32
    AF = mybir.ActivationFunctionType

    nc = bacc.Bacc("TRN2", target_bir_lowering=False, debug=False,
                   enable_asserts=False, num_devices=NCORES)
    im2col_d = nc.dram_tensor("im2col", [K1, POS1], bf16, kind="ExternalInput")
    w1t_d = nc.dram_tensor("w1t", [K1, 256], bf16, kind="ExternalInput")
    w2_d = nc.dram_tensor("w2", [2, 128, KHW * 256], bf16, kind="ExternalInput")
    uout_d = nc.dram_tensor("uout", [2, 128, BL * NPOS2], f32, kind="ExternalOutput")

    with tile.TileContext(nc) as tc:
        with tc.tile_pool(name="const", bufs=1) as const, \
             tc.tile_pool(name="ps1", bufs=2, space="PSUM") as ps1, \
             tc.tile_pool(name="ps2", bufs=3, space="PSUM") as ps2, \
             tc.tile_pool(name="outp", bufs=3) as outp:
            im2col_sb = const.tile([K1, POS1], bf16, tag="im2col")
            nc.sync.dma_start(im2col_sb[:], im2col_d.ap()[:, :])
            w1t_sb = const.tile([K1, 256], bf16, tag="w1t")
            nc.sync.dma_start(w1t_sb[:], w1t_d.ap()[:, :])
            w2_sb = []
            for ci in range(2):
                t = const.tile([128, KHW * 256], bf16, tag=f"w2_{ci}")
                nc.sync.dma_start(t[:], w2_d.ap()[ci])
                w2_sb.append(t)
            x1 = [const.tile([128, POS1], bf16, tag=f"x1_{ot}") for ot in range(2)]

            # conv1 + relu: out[oc, (img,oh,ow)] = relu(W1.T @ im2col)
            for ot in range(2):
                for c in range(POS1 // 512):
                    ps = ps1.tile([128, 512], f32, tag="c1")
                    nc.tensor.matmul(
                        ps[:], w1t_sb[:, ot * 128:(ot + 1) * 128],
                        im2col_sb[:, c * 512:(c + 1) * 512],
                        start=True, stop=True)
                    nc.scalar.activation(
                        x1[ot][:, c * 512:(c + 1) * 512], ps[:], AF.Relu)

            # primary caps conv: stride 2, 9x9, 256->256, accumulate 162 matmuls
            x1v = [x1[ot][:].rearrange("p (b h w) -> p b h w", b=BL, h=20, w=20)
                   for ot in range(2)]
            for ot in range(2):
                pss = []
                for (b0, nb) in CHUNKS:
                    pss.append(ps2.tile([128, nb * NPOS2], f32, tag="c2"))
                nk = 0
                for kh in range(9):
                    for kw in range(9):
                        for ci in range(2):
                            khkw = kh * 9 + kw
                            lhsT = w2_sb[ci][:, khkw * 256 + ot * 128:
                                             khkw * 256 + ot * 128 + 128]
                            for ic, (b0, nb) in enumerate(CHUNKS):
                                rhs = x1v[ci][:, b0:b0 + nb,
                                              kh:kh + 11:2, kw:kw + 11:2]
                                nc.tensor.matmul(pss[ic][:], lhsT, rhs,
                                                 start=(nk == 0), stop=(nk == 161))
                            nk += 1
                for ic, (b0, nb) in enumerate(CHUNKS):
                    ob = outp.tile([128, nb * NPOS2], f32, tag="ob")
                    nc.scalar.activation(ob[:], pss[ic][:], AF.Copy)
                    nc.sync.dma_start(
                        uout_d.ap()[ot][:, b0 * NPOS2:(b0 + nb) * NPOS2], ob[:])

    nc.compile()
    in_maps = [{"im2col": im2col_np[c], "w1t": w1t_np, "w2": w2_np}
               for c in range(NCORES)]
    res = run_bass_kernel_spmd(nc, in_maps, core_ids=list(range(NCORES)))
    global _exec_time_ns
    _exec_time_ns = res.exec_time_ns
    return [res.results[c]["uout"].astype(np.float32) for c in range(NCORES)]


def _host_conv_fallback(im2col_np, w1t_np, w2_np):
    outs = []
    for c in range(NCORES):
        a = im2col_np[c].astype(np.float32)          # [82, POS1]
        w1 = w1t_np.astype(np.float32)               # [82, 256]
        x1 = np.maximum(w1.T @ a, 0.0)               # [256, POS1]
        x1 = x1.reshape(256, BL, 20, 20)
        w2 = w2_np.astype(np.float32).reshape(256, KHW, 256)  # [i, khkw, o]
        acc = np.zeros((256, BL * NPOS2), np.float32)
        patches = np.empty((256 * KHW, BL * NPOS2), np.float32)
        for kh in range(9):
            for kw in range(9):
                khkw = kh * 9 + kw
                patches[khkw * 256:(khkw + 1) * 256] = (
                    x1[:, :, kh:kh + 11:2, kw:kw + 11:2].reshape(256, -1))
        wfull = w2.transpose(1, 0, 2).reshape(KHW * 256, 256)  # [(khkw,i), o]
        pf = patches.reshape(KHW, 256, -1).reshape(KHW * 256, -1)
        acc = wfull.T @ pf
        outs.append(acc.reshape(2, 128, BL * NPOS2))
    return outs


def kernel(images, labels, conv1_w, conv1_b, prim_w, prim_b, W):
    images = np.asarray(images, np.float32)
    conv1_w = np.asarray(conv1_w, np.float32)
    conv1_b = np.asarray(conv1_b, np.float32)
    prim_w = np.asarray(prim_w, np.float32)
    prim_b = np.asarray(prim_b, np.float32)
    W = np.asarray(W, np.float32)

    # host staging: im2col per core, transposed weights, all bf16
    im2col_np = []
    for c in range(NCORES):
        img = images[c * BL:(c + 1) * BL, 0]                   # [32,28,28]
        sw = np.lib.stride_tricks.sliding_window_view(img, (9, 9), axis=(1, 2))
        a = sw.transpose(3, 4, 0, 1, 2).reshape(KHW, POS1)     # [81, POS1]
        a = np.concatenate([a, np.ones((1, POS1), np.float32)], 0)
        im2col_np.append(a.astype(ml_dtypes.bfloat16))
    w1t = np.concatenate([conv1_w.reshape(256, KHW).T, conv1_b[None, :]], 0)
    w1t_np = w1t.astype(ml_dtypes.bfloat16)
    w2_np = prim_w.reshape(256, 256, KHW).transpose(1, 2, 0) \
        .reshape(2, 128, KHW * 256).astype(ml_dtypes.bfloat16)

    try:
        uouts = _build_and_run_device(im2col_np, w1t_np, w2_np)
    except Exception as e:
        import traceback
        traceback.print_exc()
        print("DEVICE PATH FAILED — numpy fallback:", e)
        uouts = _host_conv_fallback(im2col_np, w1t_np, w2_np)

    # host epilogue (exact reference math, f32)
    us = []
    for c in range(NCORES):
        y = uouts[c].reshape(256, BL, NPOS2) + prim_b[:, None, None]
        u = y.reshape(8, 32, BL, NPOS2).transpose(2, 0, 1, 3).reshape(BL, 8, 1152)
        us.append(u)
    u = np.concatenate(us, 0).transpose(0, 2, 1)               # [B,1152,8]

    sq = np.sum(u * u, axis=1, keepdims=True)                  # [B,1,8]
    u = sq / (1.0 + sq) * (u / np.sqrt(sq))
    # u_hat[b,r,j,d]
    u_hat = np.einsum('rjdi,bri->brjd', W, u, optimize=True).astype(np.float32)
    b_ij = np.zeros((1152, 10), np.float32)
    for _ in range(3):
        e = np.exp(b_ij - b_ij.max(axis=1, keepdims=True))
        c_ij = e / e.sum(axis=1, keepdims=True)
        s_j = np.einsum('rj,brjd->bjd', c_ij, u_hat, optimize=True)
        sq2 = np.sum(s_j * s_j, axis=2, keepdims=True)
        v_j = sq2 / (1.0 + sq2) * (s_j / np.sqrt(sq2))
        agree = np.einsum('brjd,bjd->brj', u_hat, v_j, optimize=True).mean(axis=0)
        b_ij = b_ij + agree
    return v_j[..., None].astype(np.float32)

